# revision 11
# baseline (speedup 1.0000x reference)
"""Trainium2 Bass kernel for nn_LiquidS4Layer (S4 DPLR forward).

y = causal_conv(u, K) + D*u, with K the length-L SSM kernel computed from
small DPLR params (Lambda, P, B, C, step).

The tiny parameter pipeline (N=64 modes -> K and the chunk-recurrence
matrices, O(N^2 L) flops) runs on host in fp64 numpy; the memory-bound
convolution over u (BH*L = 16M elements) runs on the NeuronCores.

Device algorithm, per core over 512 of the 4096 batch rows, chunk Q=128:
  near field   y[i] += T0^T u[i]          (intra-chunk causal Toeplitz + D)
  direct       y[2k+1] += G0^T u[2k]      (adjacent-chunk Toeplitz block)
  far field    y[2k] += Wout^T h_k ; y[2k+1] += W1^T h_k
  recurrence   h_k = Phi2^T h_{k-1} + E^T u[2k-2] + Min^T u[2k-1]
with h the 2N=128-dim real-embedded SSM state per row.  All matmuls are
bf16 with fp32 PSUM accumulation, 512-wide moving operands (4 row-blocks
of 128 at a time).  u arrives host-transposed/bf16 so chunk operands are
contiguous [q, rows] tiles; y leaves in bf16 chunk-major layout and is
re-assembled on host.

Sharding: u/y row-sharded over 8 cores (batch*channel parallel); the small
weight pack (7 x [128,128] bf16) is replicated; no collectives.
"""
import os
import numpy as np
import ml_dtypes
from contextlib import ExitStack

import concourse.bass as bass
import concourse.tile as tile
from concourse import mybir
from concourse.bass_utils import run_bass_kernel_spmd

F32 = mybir.dt.float32
BF16 = mybir.dt.bfloat16
NPBF16 = ml_dtypes.bfloat16

NCORES = 8
BH, L = 4096, 4096
BC = BH // NCORES       # 512 rows per core
N = 64                  # SSM state size
Q = 128                 # chunk length
NCH = L // Q            # 32 chunks
NPAIR = NCH // 2        # 16 chunk pairs

LAST_EXEC_NS = None
LAST_RESULTS = None


# --------------------------------------------------------------------------
# Host parameter pipeline (fp64): DPLR params -> K -> device weight pack
# --------------------------------------------------------------------------
def _host_weights(Lambda_re, Lambda_im, P_re, P_im, B_re, B_im, C_ri, D,
                  log_step):
    Lam = (np.asarray(Lambda_re, np.float64)
           + 1j * np.asarray(Lambda_im, np.float64)).reshape(N)
    P = (np.asarray(P_re, np.float64)
         + 1j * np.asarray(P_im, np.float64)).reshape(N)
    B = (np.asarray(B_re, np.float64)
         + 1j * np.asarray(B_im, np.float64)).reshape(N)
    C_ri = np.asarray(C_ri, np.float64).reshape(N, 2)
    C = C_ri[:, 0] + 1j * C_ri[:, 1]
    step = float(np.exp(np.asarray(log_step, np.float64).reshape(())))
    Dv = float(np.asarray(D, np.float64).reshape(()))

    # K via the reference's generating-function path (roots of unity + ifft)
    l = np.arange(L)
    Om = np.exp((-2j * np.pi) * (l / L))
    a0, a1 = np.conj(C), np.conj(P)
    b0, b1 = B, P
    g = (2.0 / step) * ((1.0 - Om) / (1.0 + Om))
    cc = 2.0 / (1.0 + Om)

    def cauchy(v):
        return (v[None, :] / (g[:, None] - Lam[None, :])).sum(-1)

    k00 = cauchy(a0 * b0)
    k01 = cauchy(a0 * b1)
    k10 = cauchy(a1 * b0)
    k11 = cauchy(a1 * b1)
    at_roots = cc * (k00 - k01 * (1.0 / (1.0 + k11)) * k10)
    K = np.fft.ifft(at_roots, L).real  # (L,) aliased causal kernel

    # State space: A = diag(Lam) - P P^H, bilinear discretization, and the
    # alias-corrected input vector Bp so that K[l] = Re(Ct @ Abar^l @ Bp).
    A = np.diag(Lam) - np.outer(P, np.conj(P))
    I = np.eye(N)
    inv = np.linalg.inv(I - (step / 2.0) * A)
    Abar = inv @ (I + (step / 2.0) * A)
    Bbar = inv @ (step * B)
    AL = np.linalg.matrix_power(Abar, L)
    Bp = np.linalg.solve(I - AL, Bbar)
    Ct = np.conj(C)

    # complex [hr; hi] block embedding
    def embed_mat(M):
        return np.block([[M.real, -M.imag], [M.imag, M.real]])

    def embed_vec(x):
        return np.concatenate([x.real, x.imag])

    A128 = np.linalg.matrix_power(Abar, 128)

    # Wout[s, t]: y_t = Re(Ct A^t h);  W1 continues t in [128, 256)
    Wout = np.zeros((2 * N, Q))
    W1 = np.zeros((2 * N, Q))
    gt = Ct.copy()
    for t in range(Q):
        Wout[:N, t] = gt.real
        Wout[N:, t] = -gt.imag
        gt = gt @ Abar
    for t in range(Q):
        W1[:N, t] = gt.real
        W1[N:, t] = -gt.imag
        gt = gt @ Abar

    # Min[s', q] = embed(A^{128-q} Bp);  E[s', q] = embed(A^{256-q} Bp)
    cols = [None] * 257  # cols[e] = A^e Bp
    v = Abar @ Bp
    for e in range(1, 257):
        cols[e] = v
        v = Abar @ v
    Min_r = np.zeros((2 * N, Q))
    E_r = np.zeros((2 * N, Q))
    for q in range(Q):
        Min_r[:, q] = embed_vec(cols[128 - q])
        E_r[:, q] = embed_vec(cols[256 - q])

    Phi2 = embed_mat(A128 @ A128)  # A^256

    # Toeplitz slabs from K (lhsT layout [q, t])
    idx_t = np.arange(Q)[None, :]
    idx_q = np.arange(Q)[:, None]
    lag = idx_t - idx_q
    T0 = np.where(lag >= 0, K[np.clip(lag, 0, L - 1)], 0.0)
    T0 = T0 + Dv * np.eye(Q)
    G0 = K[128 + lag]

    # pack, lhsT convention (partition dim = contraction dim)
    pack = np.concatenate(
        [T0, G0, Wout, W1, Min_r.T, E_r.T, Phi2.T], axis=1)  # [128, 7*128]
    return np.ascontiguousarray(pack).astype(NPBF16)


# --------------------------------------------------------------------------
# Device program
# --------------------------------------------------------------------------
def build_program():
    nc = bass.Bass()
    dp = nc.declare_dram_parameter
    uT_d = dp("uT", [128, NCH * 512], BF16, isOutput=False)
    w_d = dp("W", [128, 7 * 128], BF16, isOutput=False)
    y_d = dp("y", [NCH * 128, 512], BF16, isOutput=True)
    with TileKernel(nc) as tk:
        tk.build(uT_d, w_d, y_d)
    _split_multi_waits(nc)
    return nc


def _split_multi_waits(nc):
    """This toolchain's walrus encodes at most one sync wait per (non-Drain)
    instruction.  Tile can emit several; hoist the extras onto standalone
    EventSemaphore wait instructions inserted just before, on the same
    engine (engines execute their stream in order, so this is equivalent)."""
    ctr = 0
    for f in nc.m.functions:
        for blk in f.blocks:
            out = []
            changed = False
            for inst in blk.instructions:
                si = inst.sync_info
                if si is None:
                    out.append(inst)
                    continue
                waits = list(si.on_wait)
                if len(waits) > 1:
                    # pick a non-DMA sem for the no-op update (the sim
                    # forbids foreign updates of in-flight DMA sems)
                    cands = [u for u in si.on_update] + [
                        w for w in waits if "DMA" not in w.ant_name]
                    for w in waits[:-1]:
                        ev = mybir.InstEventSemaphore(
                            name=f"I-wsplit-{ctr}", ins=[], outs=[])
                        ctr += 1
                        ev.engine = inst.engine
                        # zero-increment update: the sim requires >=1 update
                        # per instruction; +0 changes no semaphore value.
                        c = cands[0] if cands else w
                        up = mybir.SyncUpdate(
                            sync_type="semaphore", id=c.id, ant_name=c.ant_name,
                            update_mode="sem-add-imm", update_value=0,
                            update_reg=None)
                        ev.sync_info = mybir.SyncInfo(on_wait=[w], on_update=[up])
                        out.append(ev)
                    inst.sync_info = mybir.SyncInfo(
                        on_wait=[waits[-1]], on_update=list(si.on_update))
                    changed = True
                out.append(inst)
            if changed:
                blk.instructions = out
    return nc


class TileKernel:
    def __init__(self, nc):
        self.nc = nc
        self.ctx = ExitStack()
        self.tc = tile.TileContext(nc)

    def __enter__(self):
        self.ctx.__enter__()
        self.tc.__enter__()
        return self

    def __exit__(self, *a):
        self.ctx.__exit__(*a)   # release pools before the scheduler runs
        return self.tc.__exit__(*a)

    def pool(self, name, bufs=1, space="SBUF"):
        return self.ctx.enter_context(
            self.tc.tile_pool(name=name, bufs=bufs, space=space))

    def build(self, uT_d, w_d, y_d):
        nc = self.nc
        mm = nc.tensor.matmul
        v = nc.vector
        s = nc.scalar

        wp = self.pool("w", 1)
        up = self.pool("u", 1)
        hlp = self.pool("hL", 2)
        hrp = self.pool("hR", 2)
        yp = self.pool("yt", 6)
        pyp = self.pool("py", 4, "PSUM")
        phlp = self.pool("phL", 2, "PSUM")
        phrp = self.pool("phR", 2, "PSUM")

        Wt = wp.tile([128, 7 * 128], BF16, tag="Wt", name="Wt")
        nc.sync.dma_start(out=Wt[:], in_=w_d[:])
        T0 = Wt[:, 0 * 128:1 * 128]
        G0 = Wt[:, 1 * 128:2 * 128]
        Wo = Wt[:, 2 * 128:3 * 128]
        W1 = Wt[:, 3 * 128:4 * 128]
        Mn = Wt[:, 4 * 128:5 * 128]
        Et = Wt[:, 5 * 128:6 * 128]
        Ph = Wt[:, 6 * 128:7 * 128]

        uT = up.tile([128, NCH, 4, 128], BF16, tag="uT", name="uT")
        # chunks 0-1 first (small) so the first near-field mm starts early;
        # spread dma_start issue across the three DGE-capable engines so
        # descriptor generation is not serialized on one sequencer.
        issuers = [nc.scalar, nc.sync, nc.gpsimd]
        nc.scalar.dma_start(out=uT[:, 0:2, :, :], in_=uT_d[:, 0:1024])
        nc.gpsimd.dma_start(out=uT[:, 2:4, :, :], in_=uT_d[:, 1024:2048])
        for blk in range(1, 8):
            issuers[blk % 3].dma_start(
                out=uT[:, blk * 4:(blk + 1) * 4, :, :],
                in_=uT_d[:, blk * 2048:(blk + 1) * 2048])

        def uch(i):
            return uT[:, i, :, :]

        def uch2(i, half):  # 256-wide half of a chunk (j-blocks 2h..2h+1)
            return uT[:, i, 2 * half:2 * half + 2, :]

        def emit_y(py, i, nm, eng):
            yt = yp.tile([128, 512], BF16, tag="yt", name=nm)
            if eng is v:
                v.tensor_copy(yt[:], py[:])
            else:
                s.copy(yt[:], py[:])
            nc.sync.dma_start(out=y_d[i * 128:(i + 1) * 128, :], in_=yt[:])

        # PE warm-up: a few throwaway mms on the weight tile bridge the gap
        # between Wt arriving and the first u chunks, starting the p-state
        # ramp early.  Results land in a py-pool bank that is reused later.
        warm = pyp.tile([128, 512], F32, tag="py", name="warm")
        for _ in range(3):
            mm(warm[:], T0, Wt[:, 0:512], start=True, stop=True)

        # ---- software pipeline over 16 chunk pairs -------------------
        # The serial state chain h_k -> h_{k+1} is split into two
        # independent half-chains over row halves (columns 0:256 / 256:512)
        # whose PSUM->SBUF copies run on different engines (scalar/vector),
        # halving the per-step latency so the chain stays off the critical
        # path.  The u-projection parts (Et/Mn) issue early; Phi2*h last.
        # pair 0: near fields only (h_0 = 0)
        py_a = pyp.tile([128, 512], F32, tag="py", name="py_a")
        py_b = pyp.tile([128, 512], F32, tag="py", name="py_b")
        mm(py_a[:], T0, uch(0), start=True, stop=True)
        mm(py_b[:], T0, uch(1), start=True, stop=False)
        mm(py_b[:], G0, uch(0), start=False, stop=True)
        emit_y(py_a, 0, "yt_a", v)
        emit_y(py_b, 1, "yt_b", s)
        # h_1 = Et u0 + Mn u1 (no Phi term)
        phL = phlp.tile([128, 256], F32, tag="phL", name="phL")[:]
        phR = phrp.tile([128, 256], F32, tag="phR", name="phR")[:]
        mm(phL, Et, uch2(0, 0), start=True, stop=False)
        mm(phR, Et, uch2(0, 1), start=True, stop=False)
        mm(phL, Mn, uch2(1, 0), start=False, stop=True)
        mm(phR, Mn, uch2(1, 1), start=False, stop=True)
        hL = hlp.tile([128, 256], BF16, tag="hL", name="hL")
        s.copy(hL[:], phL)
        hR = hrp.tile([128, 256], BF16, tag="hR", name="hR")
        v.tensor_copy(hR[:], phR)
        # near fields of pair 1
        py_a = pyp.tile([128, 512], F32, tag="py", name="py_a")
        py_b = pyp.tile([128, 512], F32, tag="py", name="py_b")
        mm(py_a[:], T0, uch(2), start=True, stop=False)
        mm(py_b[:], T0, uch(3), start=True, stop=False)
        mm(py_b[:], G0, uch(2), start=False, stop=True)

        for k in range(1, NPAIR):
            hL_k, hR_k = hL, hR
            last = (k == NPAIR - 1)
            if not last:
                phL = phlp.tile([128, 256], F32, tag="phL", name="phL")[:]
                phR = phrp.tile([128, 256], F32, tag="phR", name="phR")[:]
                mm(phL, Et, uch2(2 * k, 0), start=True, stop=False)
                mm(phR, Et, uch2(2 * k, 1), start=True, stop=False)
                mm(phL, Mn, uch2(2 * k + 1, 0), start=False, stop=False)
                mm(phR, Mn, uch2(2 * k + 1, 1), start=False, stop=False)
                # chain-critical: advance the state before the far fields
                mm(phL, Ph, hL_k[:], start=False, stop=True)
                mm(phR, Ph, hR_k[:], start=False, stop=True)
                hL = hlp.tile([128, 256], BF16, tag="hL", name="hL")
                s.copy(hL[:], phL)
                hR = hrp.tile([128, 256], BF16, tag="hR", name="hR")
                v.tensor_copy(hR[:], phR)
            # far fields of pair k (need h_k)
            mm(py_a[:, 0:256], Wo, hL_k[:], start=False, stop=True)
            mm(py_a[:, 256:512], Wo, hR_k[:], start=False, stop=True)
            emit_y(py_a, 2 * k, "yt_a", v)
            mm(py_b[:, 0:256], W1, hL_k[:], start=False, stop=True)
            mm(py_b[:, 256:512], W1, hR_k[:], start=False, stop=True)
            emit_y(py_b, 2 * k + 1, "yt_b", s)
            if not last:
                # near fields of pair k+1
                py_a = pyp.tile([128, 512], F32, tag="py", name="py_a")
                py_b = pyp.tile([128, 512], F32, tag="py", name="py_b")
                mm(py_a[:], T0, uch(2 * k + 2), start=True,
                   stop=False)
                mm(py_b[:], T0, uch(2 * k + 3), start=True, stop=False)
                mm(py_b[:], G0, uch(2 * k + 2), start=False, stop=True)


# --------------------------------------------------------------------------
# Entry point
# --------------------------------------------------------------------------
def kernel(**inputs):
    global LAST_EXEC_NS, LAST_RESULTS
    nc = build_program()

    W_pack = _host_weights(
        inputs["Lambda_re"], inputs["Lambda_im"], inputs["P_re"],
        inputs["P_im"], inputs["B_re"], inputs["B_im"], inputs["C_ri"],
        inputs["D"], inputs["log_step"])

    # u [BH, L] -> per-core [q, i, j, b'] bf16: uT[c, q, i*512 + j*128 + b']
    #   = u[c*512 + j*128 + b', i*128 + q]
    u = np.asarray(inputs["u"], dtype=np.float32)
    uT = np.ascontiguousarray(
        u.reshape(NCORES, 4, 128, NCH, 128).transpose(0, 4, 3, 1, 2)
    ).reshape(NCORES, 128, NCH * 512).astype(NPBF16)

    in_maps = []
    for c in range(NCORES):
        in_maps.append({"uT": uT[c], "W": W_pack})

    trace = bool(int(os.environ.get("KERNEL_TRACE", "0")))
    kw = {}
    if trace:
        kw["trace"] = True
        kw["trace_cores"] = list(range(NCORES))
    res = run_bass_kernel_spmd(nc, in_maps, list(range(NCORES)), **kw)
    LAST_EXEC_NS = res.exec_time_ns
    LAST_RESULTS = res

    # y_d rows i*128+t, cols (j, b') -> y[c, j*128+b', i*128+t]
    outs = []
    for c in range(NCORES):
        yc = np.asarray(res.results[c]["y"])  # (4096, 512) bf16
        yc = yc.reshape(NCH, 128, 4, 128).transpose(2, 3, 0, 1)
        outs.append(yc.reshape(BC, L).astype(np.float32))
    return np.concatenate(outs, axis=0)


# revision 13
# speedup vs baseline: 1.0318x; 1.0318x over previous
"""Trainium2 Bass kernel for nn_LiquidS4Layer (S4 DPLR forward).

y = causal_conv(u, K) + D*u, with K the length-L SSM kernel computed from
small DPLR params (Lambda, P, B, C, step).

The tiny parameter pipeline (N=64 modes -> K and the chunk-recurrence
matrices, O(N^2 L) flops) runs on host in fp64 numpy; the memory-bound
convolution over u (BH*L = 16M elements) runs on the NeuronCores.

Device algorithm, per core over 512 of the 4096 batch rows, chunk Q=128:
  near field   y[i] += T0^T u[i]          (intra-chunk causal Toeplitz + D)
  direct       y[2k+1] += G0^T u[2k]      (adjacent-chunk Toeplitz block)
  far field    y[2k] += Wout^T h_k ; y[2k+1] += W1^T h_k
  recurrence   h_k = Phi2^T h_{k-1} + E^T u[2k-2] + Min^T u[2k-1]
with h the 2N=128-dim real-embedded SSM state per row.  All matmuls are
bf16 with fp32 PSUM accumulation, 512-wide moving operands (4 row-blocks
of 128 at a time).  u arrives host-transposed/bf16 so chunk operands are
contiguous [q, rows] tiles; y leaves in bf16 chunk-major layout and is
re-assembled on host.

Sharding: u/y row-sharded over 8 cores (batch*channel parallel); the small
weight pack (7 x [128,128] bf16) is replicated; no collectives.
"""
import os
import numpy as np
import ml_dtypes
from contextlib import ExitStack

import concourse.bass as bass
import concourse.tile as tile
from concourse import mybir
from concourse.bass_utils import run_bass_kernel_spmd

F32 = mybir.dt.float32
BF16 = mybir.dt.bfloat16
NPBF16 = ml_dtypes.bfloat16

NCORES = 8
BH, L = 4096, 4096
BC = BH // NCORES       # 512 rows per core
N = 64                  # SSM state size
Q = 128                 # chunk length
NCH = L // Q            # 32 chunks
NPAIR = NCH // 2        # 16 chunk pairs

LAST_EXEC_NS = None
LAST_RESULTS = None


# --------------------------------------------------------------------------
# Host parameter pipeline (fp64): DPLR params -> K -> device weight pack
# --------------------------------------------------------------------------
def _host_weights(Lambda_re, Lambda_im, P_re, P_im, B_re, B_im, C_ri, D,
                  log_step):
    Lam = (np.asarray(Lambda_re, np.float64)
           + 1j * np.asarray(Lambda_im, np.float64)).reshape(N)
    P = (np.asarray(P_re, np.float64)
         + 1j * np.asarray(P_im, np.float64)).reshape(N)
    B = (np.asarray(B_re, np.float64)
         + 1j * np.asarray(B_im, np.float64)).reshape(N)
    C_ri = np.asarray(C_ri, np.float64).reshape(N, 2)
    C = C_ri[:, 0] + 1j * C_ri[:, 1]
    step = float(np.exp(np.asarray(log_step, np.float64).reshape(())))
    Dv = float(np.asarray(D, np.float64).reshape(()))

    # K via the reference's generating-function path (roots of unity + ifft)
    l = np.arange(L)
    Om = np.exp((-2j * np.pi) * (l / L))
    a0, a1 = np.conj(C), np.conj(P)
    b0, b1 = B, P
    g = (2.0 / step) * ((1.0 - Om) / (1.0 + Om))
    cc = 2.0 / (1.0 + Om)

    def cauchy(v):
        return (v[None, :] / (g[:, None] - Lam[None, :])).sum(-1)

    k00 = cauchy(a0 * b0)
    k01 = cauchy(a0 * b1)
    k10 = cauchy(a1 * b0)
    k11 = cauchy(a1 * b1)
    at_roots = cc * (k00 - k01 * (1.0 / (1.0 + k11)) * k10)
    K = np.fft.ifft(at_roots, L).real  # (L,) aliased causal kernel

    # State space: A = diag(Lam) - P P^H, bilinear discretization, and the
    # alias-corrected input vector Bp so that K[l] = Re(Ct @ Abar^l @ Bp).
    A = np.diag(Lam) - np.outer(P, np.conj(P))
    I = np.eye(N)
    inv = np.linalg.inv(I - (step / 2.0) * A)
    Abar = inv @ (I + (step / 2.0) * A)
    Bbar = inv @ (step * B)
    AL = np.linalg.matrix_power(Abar, L)
    Bp = np.linalg.solve(I - AL, Bbar)
    Ct = np.conj(C)

    # complex [hr; hi] block embedding
    def embed_mat(M):
        return np.block([[M.real, -M.imag], [M.imag, M.real]])

    def embed_vec(x):
        return np.concatenate([x.real, x.imag])

    A128 = np.linalg.matrix_power(Abar, 128)

    # Wout[s, t]: y_t = Re(Ct A^t h);  W1 continues t in [128, 256)
    Wout = np.zeros((2 * N, Q))
    W1 = np.zeros((2 * N, Q))
    gt = Ct.copy()
    for t in range(Q):
        Wout[:N, t] = gt.real
        Wout[N:, t] = -gt.imag
        gt = gt @ Abar
    for t in range(Q):
        W1[:N, t] = gt.real
        W1[N:, t] = -gt.imag
        gt = gt @ Abar

    # Min[s', q] = embed(A^{128-q} Bp);  E[s', q] = embed(A^{256-q} Bp)
    cols = [None] * 257  # cols[e] = A^e Bp
    v = Abar @ Bp
    for e in range(1, 257):
        cols[e] = v
        v = Abar @ v
    Min_r = np.zeros((2 * N, Q))
    E_r = np.zeros((2 * N, Q))
    for q in range(Q):
        Min_r[:, q] = embed_vec(cols[128 - q])
        E_r[:, q] = embed_vec(cols[256 - q])

    A256 = A128 @ A128
    Phi2 = embed_mat(A256)  # A^256
    # shifted projection maps for the chain-free prologue states h_2, h_3
    P2 = embed_mat(A256)
    P2E, P2M = P2 @ E_r, P2 @ Min_r
    P4 = embed_mat(A256 @ A256)
    P4E, P4M = P4 @ E_r, P4 @ Min_r

    # Toeplitz slabs from K (lhsT layout [q, t])
    idx_t = np.arange(Q)[None, :]
    idx_q = np.arange(Q)[:, None]
    lag = idx_t - idx_q
    T0 = np.where(lag >= 0, K[np.clip(lag, 0, L - 1)], 0.0)
    T0 = T0 + Dv * np.eye(Q)
    G0 = K[128 + lag]

    # pack, lhsT convention (partition dim = contraction dim)
    pack = np.concatenate(
        [T0, G0, Wout, W1, Min_r.T, E_r.T, Phi2.T,
         P2E.T, P2M.T, P4E.T, P4M.T], axis=1)  # [128, 11*128]
    return np.ascontiguousarray(pack).astype(NPBF16)


# --------------------------------------------------------------------------
# Device program
# --------------------------------------------------------------------------
def build_program():
    nc = bass.Bass()
    dp = nc.declare_dram_parameter
    uT_d = dp("uT", [128, NCH * 512], BF16, isOutput=False)
    w_d = dp("W", [128, 11 * 128], BF16, isOutput=False)
    y_d = dp("y", [NCH * 128, 512], BF16, isOutput=True)
    with TileKernel(nc) as tk:
        tk.build(uT_d, w_d, y_d)
    _split_multi_waits(nc)
    return nc


def _split_multi_waits(nc):
    """This toolchain's walrus encodes at most one sync wait per (non-Drain)
    instruction.  Tile can emit several; hoist the extras onto standalone
    EventSemaphore wait instructions inserted just before, on the same
    engine (engines execute their stream in order, so this is equivalent)."""
    ctr = 0
    for f in nc.m.functions:
        for blk in f.blocks:
            out = []
            changed = False
            for inst in blk.instructions:
                si = inst.sync_info
                if si is None:
                    out.append(inst)
                    continue
                waits = list(si.on_wait)
                if len(waits) > 1:
                    # pick a non-DMA sem for the no-op update (the sim
                    # forbids foreign updates of in-flight DMA sems)
                    cands = [u for u in si.on_update] + [
                        w for w in waits if "DMA" not in w.ant_name]
                    for w in waits[:-1]:
                        ev = mybir.InstEventSemaphore(
                            name=f"I-wsplit-{ctr}", ins=[], outs=[])
                        ctr += 1
                        ev.engine = inst.engine
                        # zero-increment update: the sim requires >=1 update
                        # per instruction; +0 changes no semaphore value.
                        c = cands[0] if cands else w
                        up = mybir.SyncUpdate(
                            sync_type="semaphore", id=c.id, ant_name=c.ant_name,
                            update_mode="sem-add-imm", update_value=0,
                            update_reg=None)
                        ev.sync_info = mybir.SyncInfo(on_wait=[w], on_update=[up])
                        out.append(ev)
                    inst.sync_info = mybir.SyncInfo(
                        on_wait=[waits[-1]], on_update=list(si.on_update))
                    changed = True
                out.append(inst)
            if changed:
                blk.instructions = out
    return nc


class TileKernel:
    def __init__(self, nc):
        self.nc = nc
        self.ctx = ExitStack()
        self.tc = tile.TileContext(nc)

    def __enter__(self):
        self.ctx.__enter__()
        self.tc.__enter__()
        return self

    def __exit__(self, *a):
        self.ctx.__exit__(*a)   # release pools before the scheduler runs
        return self.tc.__exit__(*a)

    def pool(self, name, bufs=1, space="SBUF"):
        return self.ctx.enter_context(
            self.tc.tile_pool(name=name, bufs=bufs, space=space))

    def build(self, uT_d, w_d, y_d):
        nc = self.nc
        mm = nc.tensor.matmul
        v = nc.vector
        s = nc.scalar

        wp = self.pool("w", 1)
        up = self.pool("u", 1)
        hlp = self.pool("hL", 3)
        hrp = self.pool("hR", 3)
        yp = self.pool("yt", 6)
        pyp = self.pool("py", 4, "PSUM")
        phlp = self.pool("phL", 2, "PSUM")
        phrp = self.pool("phR", 2, "PSUM")

        Wt = wp.tile([128, 11 * 128], BF16, tag="Wt", name="Wt")
        nc.sync.dma_start(out=Wt[:], in_=w_d[:])
        (T0, G0, Wo, W1, Mn, Et, Ph, P2E, P2M, P4E, P4M) = (
            Wt[:, m * 128:(m + 1) * 128] for m in range(11))

        uT = up.tile([128, NCH, 4, 128], BF16, tag="uT", name="uT")
        # chunks 0-1 first (small) so the first near-field mm starts early;
        # spread dma_start issue across the three DGE-capable engines so
        # descriptor generation is not serialized on one sequencer.
        issuers = [nc.scalar, nc.sync, nc.gpsimd]
        for j in range(4):  # chunks 0-7 in 2-chunk pieces for early start
            issuers[j % 3].dma_start(
                out=uT[:, 2 * j:2 * j + 2, :, :],
                in_=uT_d[:, j * 1024:(j + 1) * 1024])
        for blk in range(2, 8):
            issuers[blk % 3].dma_start(
                out=uT[:, blk * 4:(blk + 1) * 4, :, :],
                in_=uT_d[:, blk * 2048:(blk + 1) * 2048])

        def uch(i):
            return uT[:, i, :, :]

        def uch2(i, half):  # 256-wide half of a chunk (j-blocks 2h..2h+1)
            return uT[:, i, 2 * half:2 * half + 2, :]

        def emit_y(py, i, nm, eng):
            yt = yp.tile([128, 512], BF16, tag="yt", name=nm)
            if eng is v:
                v.tensor_copy(yt[:], py[:])
            else:
                s.copy(yt[:], py[:])
            nc.sync.dma_start(out=y_d[i * 128:(i + 1) * 128, :], in_=yt[:])

        # ---- software pipeline over 16 chunk pairs -------------------
        # The serial state chain h_k -> h_{k+1} is split into two
        # independent half-chains over row halves (columns 0:256 / 256:512)
        # whose PSUM->SBUF copies run on different engines (scalar/vector),
        # halving the per-step latency so the chain stays off the critical
        # path.  The u-projection parts (Et/Mn) issue early; Phi2*h last.
        # pair 0: near fields only (h_0 = 0)
        py_a = pyp.tile([128, 512], F32, tag="py", name="py_a")
        py_b = pyp.tile([128, 512], F32, tag="py", name="py_b")
        mm(py_a[:], T0, uch(0), start=True, stop=True)
        mm(py_b[:], T0, uch(1), start=True, stop=False)
        mm(py_b[:], G0, uch(0), start=False, stop=True)
        emit_y(py_a, 0, "yt_a", v)
        emit_y(py_b, 1, "yt_b", s)
        def rec_state(terms, ph_extra=None):
            # build a state from u-projection terms (+ optional Phi2*h last);
            # returns the bf16 (hL, hR) copies on scalar/vector.
            phL = phlp.tile([128, 256], F32, tag="phL", name="phL")[:]
            phR = phrp.tile([128, 256], F32, tag="phR", name="phR")[:]
            for m, (w, c) in enumerate(terms):
                lastm = (m == len(terms) - 1) and ph_extra is None
                mm(phL, w, uch2(c, 0), start=(m == 0), stop=lastm)
                mm(phR, w, uch2(c, 1), start=(m == 0), stop=lastm)
            if ph_extra is not None:
                hLp, hRp = ph_extra
                mm(phL, Ph, hLp[:], start=False, stop=True)
                mm(phR, Ph, hRp[:], start=False, stop=True)
            hL = hlp.tile([128, 256], BF16, tag="hL", name="hL")
            s.copy(hL[:], phL)
            hR = hrp.tile([128, 256], BF16, tag="hR", name="hR")
            v.tensor_copy(hR[:], phR)
            return hL, hR

        # h_1 = Et u0 + Mn u1 (no Phi term)
        hL, hR = rec_state([(Et, 0), (Mn, 1)])
        # near fields of pair 1
        py_a = pyp.tile([128, 512], F32, tag="py", name="py_a")
        py_b = pyp.tile([128, 512], F32, tag="py", name="py_b")
        mm(py_a[:], T0, uch(2), start=True, stop=False)
        mm(py_b[:], T0, uch(3), start=True, stop=False)
        mm(py_b[:], G0, uch(2), start=False, stop=True)

        for k in range(1, NPAIR):
            hL_k, hR_k = hL, hR
            last = (k == NPAIR - 1)
            if not last:
                # h_{k+1}: chain-free direct maps for the first two, then
                # the Phi2 chain (advanced before the far fields).
                if k == 1:
                    hL, hR = rec_state(
                        [(P2E, 0), (P2M, 1), (Et, 2), (Mn, 3)])
                elif k == 2:
                    hL, hR = rec_state(
                        [(P4E, 0), (P4M, 1), (P2E, 2), (P2M, 3),
                         (Et, 4), (Mn, 5)])
                else:
                    hL, hR = rec_state(
                        [(Et, 2 * k), (Mn, 2 * k + 1)],
                        ph_extra=(hL_k, hR_k))
            # far fields of pair k (need h_k)
            mm(py_a[:, 0:256], Wo, hL_k[:], start=False, stop=True)
            mm(py_a[:, 256:512], Wo, hR_k[:], start=False, stop=True)
            emit_y(py_a, 2 * k, "yt_a", v)
            mm(py_b[:, 0:256], W1, hL_k[:], start=False, stop=True)
            mm(py_b[:, 256:512], W1, hR_k[:], start=False, stop=True)
            emit_y(py_b, 2 * k + 1, "yt_b", s)
            if not last:
                # near fields of pair k+1
                py_a = pyp.tile([128, 512], F32, tag="py", name="py_a")
                py_b = pyp.tile([128, 512], F32, tag="py", name="py_b")
                mm(py_a[:], T0, uch(2 * k + 2), start=True,
                   stop=False)
                mm(py_b[:], T0, uch(2 * k + 3), start=True, stop=False)
                mm(py_b[:], G0, uch(2 * k + 2), start=False, stop=True)


# --------------------------------------------------------------------------
# Entry point
# --------------------------------------------------------------------------
def kernel(**inputs):
    global LAST_EXEC_NS, LAST_RESULTS
    nc = build_program()

    W_pack = _host_weights(
        inputs["Lambda_re"], inputs["Lambda_im"], inputs["P_re"],
        inputs["P_im"], inputs["B_re"], inputs["B_im"], inputs["C_ri"],
        inputs["D"], inputs["log_step"])

    # u [BH, L] -> per-core [q, i, j, b'] bf16: uT[c, q, i*512 + j*128 + b']
    #   = u[c*512 + j*128 + b', i*128 + q]
    u = np.asarray(inputs["u"], dtype=np.float32)
    uT = np.ascontiguousarray(
        u.reshape(NCORES, 4, 128, NCH, 128).transpose(0, 4, 3, 1, 2)
    ).reshape(NCORES, 128, NCH * 512).astype(NPBF16)

    in_maps = []
    for c in range(NCORES):
        in_maps.append({"uT": uT[c], "W": W_pack})

    trace = bool(int(os.environ.get("KERNEL_TRACE", "0")))
    kw = {}
    if trace:
        kw["trace"] = True
        kw["trace_cores"] = list(range(NCORES))
    res = run_bass_kernel_spmd(nc, in_maps, list(range(NCORES)), **kw)
    LAST_EXEC_NS = res.exec_time_ns
    LAST_RESULTS = res

    # y_d rows i*128+t, cols (j, b') -> y[c, j*128+b', i*128+t]
    outs = []
    for c in range(NCORES):
        yc = np.asarray(res.results[c]["y"])  # (4096, 512) bf16
        yc = yc.reshape(NCH, 128, 4, 128).transpose(2, 3, 0, 1)
        outs.append(yc.reshape(BC, L).astype(np.float32))
    return np.concatenate(outs, axis=0)
